# revision 6
# baseline (speedup 1.0000x reference)
"""Causal self-attention on 8 Trainium2 NeuronCores.

Sharding: batch (2) x head-groups (4 heads each) -> 8 cores. Each core
computes Q/K/V projections for its 4 heads, causal attention, and the
partial output projection for its head rows of Wo. The host sums the 4
partials per batch (the "all-reduce" of the row-sharded Wo done on host
during the gather step).

Device-side layout is fully transposed: QT/KT [m, s] come straight out of
W-stationary matmuls, scoresT [sk, sq] feed an augmented-V matmul whose
extra ones-column produces the softmax denominator for free, and the
normalized attendedT [m, s] is exactly the stationary operand the output
projection wants. The causal mask is applied as a multiplicative
upper-triangular 128x128 block on the diagonal score chunks; off-diagonal
masked chunks are never computed.

All matmul operands live in float32r (TF32-like, 1 PE cycle/row vs 4 for
fp32); PSUM accumulation stays fp32.
"""

from contextlib import ExitStack

import numpy as np

import concourse.bacc as bacc
import concourse.bass as bass  # noqa: F401  (AP helpers)
import concourse.mybir as mybir
import concourse.tile as tile
from concourse.bass_utils import run_bass_kernel_spmd

P = 128
B, S, D, H, HD = 2, 2048, 1024, 16, 64
NCORES = 8
HC = 4            # heads per core
MC = HC * HD      # 256 output columns (m) per core
VW = HC * (HD + 1)  # V'' width: 4 heads x (64 vals + 1 ones col)
NDC = D // P      # 8 contraction chunks
NST = S // P      # 16 sequence tiles
F32 = mybir.dt.float32
R32 = mybir.dt.float32r

_NC_CACHE = None


def _pieces(c0, c1, step=512):
    """Split [c0, c1) at `step`-aligned boundaries (PSUM-bank-safe matmuls)."""
    out = []
    c = c0
    while c < c1:
        n = min(c1, (c // step + 1) * step)
        out.append((c, n))
        c = n
    return out


def _build_program():
    nc = bacc.Bacc("TRN2", target_bir_lowering=False, debug=False)
    xt = nc.dram_tensor("xt", [D, S], R32, kind="ExternalInput").ap()
    wq = nc.dram_tensor("wq", [D, MC], R32, kind="ExternalInput").ap()
    wk = nc.dram_tensor("wk", [D, MC], R32, kind="ExternalInput").ap()
    wv = nc.dram_tensor("wv", [D, VW], R32, kind="ExternalInput").ap()
    wo = nc.dram_tensor("wo", [MC, D], R32, kind="ExternalInput").ap()
    tri = nc.dram_tensor("tri", [P, P], R32, kind="ExternalInput").ap()
    out = nc.dram_tensor("out", [S, D], F32, kind="ExternalOutput").ap()

    with tile.TileContext(nc) as tc, ExitStack() as ctx, \
            nc.allow_low_precision(reason="float32r matmul pipeline"):
        constp = ctx.enter_context(tc.tile_pool(name="constp", bufs=1))
        xtp = ctx.enter_context(tc.tile_pool(name="xtp", bufs=1))
        wp = ctx.enter_context(tc.tile_pool(name="wp", bufs=1))
        qkp = ctx.enter_context(tc.tile_pool(name="qkp", bufs=1))
        vp = ctx.enter_context(tc.tile_pool(name="vp", bufs=1))
        attp = ctx.enter_context(tc.tile_pool(name="attp", bufs=1))
        expp = ctx.enter_context(tc.tile_pool(name="expp", bufs=2))
        outp = ctx.enter_context(tc.tile_pool(name="outp", bufs=2))
        drp = ctx.enter_context(tc.tile_pool(name="drp", bufs=1))
        ps = ctx.enter_context(tc.tile_pool(name="ps", bufs=4, space="PSUM"))

        # constants: causal-keep mask (tri[r,c] = r<=c) + a ones row for the
        # denominator broadcast matmul, packed into one tile
        trio = constp.tile([P, P + 64], R32)
        nc.sync.dma_start(trio[:, 0:P], tri)
        # memset can't target f32r; write the 1.0f bit pattern as uint32
        ONE_BITS = 0x3F800000
        nc.vector.memset(trio[0:1, P:P + 64].bitcast(mybir.dt.uint32), ONE_BITS)
        tri_sb = trio[:, 0:P]
        ones_sb = trio[0:1, P:P + 64]

        xt_sb = xtp.tile([P, NDC, S], R32)
        for dc in range(NDC):
            nc.sync.dma_start(xt_sb[:, dc, :], xt[dc * P:(dc + 1) * P, :])
        wq_sb = wp.tile([P, NDC, MC], R32)
        wk_sb = wp.tile([P, NDC, MC], R32)
        wv_sb = wp.tile([P, NDC, VW], R32)
        wo_sb = wp.tile([P, 2, D], R32)
        for dc in range(NDC):
            nc.sync.dma_start(wq_sb[:, dc, :], wq[dc * P:(dc + 1) * P, :])
            nc.sync.dma_start(wk_sb[:, dc, :], wk[dc * P:(dc + 1) * P, :])
            nc.sync.dma_start(wv_sb[:, dc, :], wv[dc * P:(dc + 1) * P, :])
        for mc2 in range(2):
            nc.sync.dma_start(wo_sb[:, mc2, :], wo[mc2 * P:(mc2 + 1) * P, :])

        # ---- projections: QT/KT [m, s] (W stationary), V natural [s, m'] ----
        qt_sb = qkp.tile([P, 2, S], R32)
        kt_sb = qkp.tile([P, 2, S], R32)
        v_sb = vp.tile([P, NST, VW], R32)
        for mc2 in range(2):
            for slab in range(4):
                s0 = slab * 512
                pq = ps.tile([P, 512], F32, tag="ps")
                pk = ps.tile([P, 512], F32, tag="ps")
                for dc in range(NDC):
                    nc.tensor.matmul(pq[:, :],
                                     wq_sb[:, dc, mc2 * P:(mc2 + 1) * P],
                                     xt_sb[:, dc, s0:s0 + 512],
                                     start=(dc == 0), stop=(dc == NDC - 1))
                for dc in range(NDC):
                    nc.tensor.matmul(pk[:, :],
                                     wk_sb[:, dc, mc2 * P:(mc2 + 1) * P],
                                     xt_sb[:, dc, s0:s0 + 512],
                                     start=(dc == 0), stop=(dc == NDC - 1))
                nc.vector.tensor_copy(qt_sb[:, mc2, s0:s0 + 512], pq[:, :])
                nc.vector.tensor_copy(kt_sb[:, mc2, s0:s0 + 512], pk[:, :])
        for st in range(NST):
            pv = ps.tile([P, VW], F32, tag="ps")
            for dc in range(NDC):
                nc.tensor.matmul(pv[:, :],
                                 xt_sb[:, dc, st * P:(st + 1) * P],
                                 wv_sb[:, dc, :],
                                 start=(dc == 0), stop=(dc == NDC - 1))
            nc.vector.tensor_copy(v_sb[:, st, :], pv[:, :])
        for j in range(HC):
            nc.vector.memset(
                v_sb[:, :, j * (HD + 1) + HD].bitcast(mybir.dt.uint32), ONE_BITS)

        # ---- attention, head by head, sq split in two halves for PSUM ----
        att_sb = attp.tile([P, 2, S], R32)
        for hh in range(HC):
            mcq = hh // 2
            poff = (hh % 2) * 64
            vlo = hh * (HD + 1)
            for half in range(2):
                hbase = half * 1024
                nch = (half + 1) * 8  # causal: sk chunks 0 .. sq_max/128
                # last chunk touching each 512-col psum bank (for stop flags)
                last_t = {0: max(i for i in range(nch)
                                 if max(0, i * P - hbase) < 512),
                          1: nch - 1}
                pa = ps.tile([P, 1024], F32, tag="ps")
                for i in range(nch):
                    c0 = max(0, i * P - hbase)  # first valid sq col (local)
                    pscr = ps.tile([P, 1024], F32, tag="ps")
                    for (a, b) in _pieces(c0, 1024):
                        nc.tensor.matmul(
                            pscr[:, a:b],
                            kt_sb[poff:poff + 64, mcq, i * P:(i + 1) * P],
                            qt_sb[poff:poff + 64, mcq, hbase + a:hbase + b],
                            start=True, stop=True)
                    et = expp.tile([P, 1024], R32)
                    nc.scalar.activation(out=et[:, c0:1024],
                                         in_=pscr[:, c0:1024],
                                         func=mybir.ActivationFunctionType.Exp,
                                         scale=0.125)
                    if i * P >= hbase:  # diagonal block: zero sk > sq
                        nc.vector.tensor_mul(et[:, c0:c0 + P],
                                             et[:, c0:c0 + P], tri_sb)
                    for (a, b) in _pieces(c0, 1024):
                        nc.tensor.matmul(
                            pa[0:HD + 1, a:b],
                            v_sb[:, i, vlo:vlo + HD + 1],
                            et[:, a:b],
                            start=(i == 0), stop=(i == last_t[a // 512]))
                # normalize: row HD of pa is the softmax denominator
                dr = drp.tile([1, 2048], R32)
                nc.vector.tensor_copy(dr[:, 0:1024], pa[HD:HD + 1, :])
                nc.vector.reciprocal(dr[:, 1024:2048], dr[:, 0:1024])
                pb = ps.tile([64, 1024], F32, tag="ps")
                for (a, b) in _pieces(0, 1024):
                    nc.tensor.matmul(pb[:, a:b], ones_sb,
                                     dr[:, 1024 + a:1024 + b],
                                     start=True, stop=True)
                asl = att_sb[poff:poff + 64, mcq, hbase:hbase + 1024]
                nc.vector.tensor_copy(asl, pa[0:64, :])
                nc.vector.tensor_mul(asl, asl, pb[:, :])

        # ---- output projection: out[s, :] = attT.T @ Wo_c ----
        for st in range(NST):
            po = ps.tile([P, 1024], F32, tag="ps")
            for mc2 in range(2):
                for (a, b) in _pieces(0, 1024):
                    nc.tensor.matmul(po[:, a:b],
                                     att_sb[:, mc2, st * P:(st + 1) * P],
                                     wo_sb[:, mc2, a:b],
                                     start=(mc2 == 0), stop=(mc2 == 1))
            ot = outp.tile([P, 1024], F32)
            nc.scalar.copy(ot[:, :], po[:, :])
            nc.sync.dma_start(out[st * P:(st + 1) * P, :], ot[:, :])

    nc.compile()
    return nc


def get_program():
    global _NC_CACHE
    if _NC_CACHE is None:
        _NC_CACHE = _build_program()
    return _NC_CACHE


def prepare_in_maps(inputs):
    x = np.asarray(inputs["x"], dtype=np.float32)
    Wq = np.asarray(inputs["Wq"], dtype=np.float32)
    Wk = np.asarray(inputs["Wk"], dtype=np.float32)
    Wv = np.asarray(inputs["Wv"], dtype=np.float32)
    Wo = np.asarray(inputs["Wo"], dtype=np.float32)
    xts = [np.ascontiguousarray(x[b].T) for b in range(B)]
    tri = np.triu(np.ones((P, P), dtype=np.float32))
    in_maps = []
    for c in range(NCORES):
        b = c // 4
        hg = c % 4
        cols = slice(hg * MC, (hg + 1) * MC)
        wv_c = np.zeros((D, VW), np.float32)
        for j in range(HC):
            wv_c[:, j * (HD + 1):j * (HD + 1) + HD] = \
                Wv[:, hg * MC + j * HD:hg * MC + (j + 1) * HD]
        in_maps.append({
            "xt": xts[b],
            "wq": np.ascontiguousarray(Wq[:, cols]),
            "wk": np.ascontiguousarray(Wk[:, cols]),
            "wv": wv_c,
            "wo": np.ascontiguousarray(Wo[cols, :]),
            "tri": tri,
        })
    return in_maps


def gather_output(results):
    outs = [np.asarray(results[c]["out"], dtype=np.float32)
            for c in range(NCORES)]
    return np.stack([outs[0] + outs[1] + outs[2] + outs[3],
                     outs[4] + outs[5] + outs[6] + outs[7]])


def kernel(**inputs) -> np.ndarray:
    nc = get_program()
    in_maps = prepare_in_maps(inputs)
    res = run_bass_kernel_spmd(nc, in_maps, list(range(NCORES)))
    return gather_output(res.results)


# revision 8
# speedup vs baseline: 1.0587x; 1.0587x over previous
"""Causal self-attention on 8 Trainium2 NeuronCores.

Sharding: batch (2) x head-groups (4 heads each) -> 8 cores. Each core
computes Q/K/V projections for its 4 heads, causal attention, and the
partial output projection for its head rows of Wo. The host sums the 4
partials per batch (the "all-reduce" of the row-sharded Wo done on host
during the gather step).

Device-side layout is fully transposed: QT/KT [m, s] come straight out of
W-stationary matmuls, scoresT [sk, sq] feed an augmented-V matmul whose
extra ones-column produces the softmax denominator for free, and the
normalized attendedT [m, s] is exactly the stationary operand the output
projection wants. The causal mask is applied as a multiplicative
upper-triangular 128x128 block on the diagonal score chunks; off-diagonal
masked chunks are never computed.

All matmul operands live in float32r (TF32-like, 1 PE cycle/row vs 4 for
fp32); PSUM accumulation stays fp32.
"""

from contextlib import ExitStack

import numpy as np

import concourse.bacc as bacc
import concourse.bass as bass  # noqa: F401  (AP helpers)
import concourse.mybir as mybir
import concourse.tile as tile
from concourse.bass_utils import run_bass_kernel_spmd

P = 128
B, S, D, H, HD = 2, 2048, 1024, 16, 64
NCORES = 8
HC = 4            # heads per core
MC = HC * HD      # 256 output columns (m) per core
VW = HC * (HD + 1)  # V'' width: 4 heads x (64 vals + 1 ones col)
NDC = D // P      # 8 contraction chunks
NST = S // P      # 16 sequence tiles
F32 = mybir.dt.float32
R32 = mybir.dt.float32r

_NC_CACHE = None


def _pieces(c0, c1, step=512):
    """Split [c0, c1) at `step`-aligned boundaries (PSUM-bank-safe matmuls)."""
    out = []
    c = c0
    while c < c1:
        n = min(c1, (c // step + 1) * step)
        out.append((c, n))
        c = n
    return out


def _build_program():
    nc = bacc.Bacc("TRN2", target_bir_lowering=False, debug=False)
    xt = nc.dram_tensor("xt", [D, S], R32, kind="ExternalInput").ap()
    wq = nc.dram_tensor("wq", [D, MC], R32, kind="ExternalInput").ap()
    wk = nc.dram_tensor("wk", [D, MC], R32, kind="ExternalInput").ap()
    wv = nc.dram_tensor("wv", [D, VW], R32, kind="ExternalInput").ap()
    wo = nc.dram_tensor("wo", [MC, D], R32, kind="ExternalInput").ap()
    tri = nc.dram_tensor("tri", [P, P], R32, kind="ExternalInput").ap()
    out = nc.dram_tensor("out", [S, D], F32, kind="ExternalOutput").ap()

    with tile.TileContext(nc) as tc, ExitStack() as ctx, \
            nc.allow_low_precision(reason="float32r matmul pipeline"):
        constp = ctx.enter_context(tc.tile_pool(name="constp", bufs=1))
        xtp = ctx.enter_context(tc.tile_pool(name="xtp", bufs=1))
        wp = ctx.enter_context(tc.tile_pool(name="wp", bufs=1))
        qkp = ctx.enter_context(tc.tile_pool(name="qkp", bufs=1))
        vp = ctx.enter_context(tc.tile_pool(name="vp", bufs=1))
        attp = ctx.enter_context(tc.tile_pool(name="attp", bufs=1))
        expp = ctx.enter_context(tc.tile_pool(name="expp", bufs=2))
        outp = ctx.enter_context(tc.tile_pool(name="outp", bufs=2))
        drp = ctx.enter_context(tc.tile_pool(name="drp", bufs=1))
        ps = ctx.enter_context(tc.tile_pool(name="ps", bufs=4, space="PSUM"))

        # constants: causal-keep mask (tri[r,c] = r<=c) + a ones row for the
        # denominator broadcast matmul, packed into one tile
        trio = constp.tile([P, P + 64], R32)
        nc.sync.dma_start(trio[:, 0:P], tri)
        # memset can't target f32r; write the 1.0f bit pattern as uint32
        ONE_BITS = 0x3F800000
        nc.vector.memset(trio[0:1, P:P + 64].bitcast(mybir.dt.uint32), ONE_BITS)
        tri_sb = trio[:, 0:P]
        ones_sb = trio[0:1, P:P + 64]

        # weights first (small), then x chunks — lets the first QKV matmuls
        # start as soon as xt chunk 0 lands instead of after the full 10.9MB
        wq_sb = wp.tile([P, NDC, MC], R32)
        wk_sb = wp.tile([P, NDC, MC], R32)
        wv_sb = wp.tile([P, NDC, VW], R32)
        wo_sb = wp.tile([P, 2, D], R32)
        for dc in range(NDC):
            nc.sync.dma_start(wq_sb[:, dc, :], wq[dc * P:(dc + 1) * P, :])
            nc.sync.dma_start(wk_sb[:, dc, :], wk[dc * P:(dc + 1) * P, :])
            nc.sync.dma_start(wv_sb[:, dc, :], wv[dc * P:(dc + 1) * P, :])
        xt_sb = xtp.tile([P, NDC, S], R32)
        for dc in range(NDC):
            nc.sync.dma_start(xt_sb[:, dc, :], xt[dc * P:(dc + 1) * P, :])
        for mc2 in range(2):
            nc.sync.dma_start(wo_sb[:, mc2, :], wo[mc2 * P:(mc2 + 1) * P, :])

        # ---- projections: QT/KT [m, s] (W stationary), V natural [s, m'] ----
        qt_sb = qkp.tile([P, 2, S], R32)
        kt_sb = qkp.tile([P, 2, S], R32)
        v_sb = vp.tile([P, NST, VW], R32)
        for mc2 in range(2):
            for slab in range(4):
                s0 = slab * 512
                pq = ps.tile([P, 512], F32, tag="ps")
                pk = ps.tile([P, 512], F32, tag="ps")
                for dc in range(NDC):
                    nc.tensor.matmul(pq[:, :],
                                     wq_sb[:, dc, mc2 * P:(mc2 + 1) * P],
                                     xt_sb[:, dc, s0:s0 + 512],
                                     start=(dc == 0), stop=(dc == NDC - 1))
                for dc in range(NDC):
                    nc.tensor.matmul(pk[:, :],
                                     wk_sb[:, dc, mc2 * P:(mc2 + 1) * P],
                                     xt_sb[:, dc, s0:s0 + 512],
                                     start=(dc == 0), stop=(dc == NDC - 1))
                nc.vector.tensor_copy(qt_sb[:, mc2, s0:s0 + 512], pq[:, :])
                nc.vector.tensor_copy(kt_sb[:, mc2, s0:s0 + 512], pk[:, :])
        for st in range(NST):
            pv = ps.tile([P, VW], F32, tag="ps")
            for dc in range(NDC):
                nc.tensor.matmul(pv[:, :],
                                 xt_sb[:, dc, st * P:(st + 1) * P],
                                 wv_sb[:, dc, :],
                                 start=(dc == 0), stop=(dc == NDC - 1))
            nc.vector.tensor_copy(v_sb[:, st, :], pv[:, :])
        for j in range(HC):
            nc.vector.memset(
                v_sb[:, :, j * (HD + 1) + HD].bitcast(mybir.dt.uint32), ONE_BITS)

        # ---- attention, head by head, sq split in two halves for PSUM ----
        att_sb = attp.tile([P, 2, S], R32)
        for hh in range(HC):
            mcq = hh // 2
            poff = (hh % 2) * 64
            vlo = hh * (HD + 1)
            for half in range(2):
                hbase = half * 1024
                nch = (half + 1) * 8  # causal: sk chunks 0 .. sq_max/128
                # last chunk touching each 512-col psum bank (for stop flags)
                last_t = {0: max(i for i in range(nch)
                                 if max(0, i * P - hbase) < 512),
                          1: nch - 1}
                pa = ps.tile([P, 1024], F32, tag="ps")
                for i in range(nch):
                    c0 = max(0, i * P - hbase)  # first valid sq col (local)
                    pscr = ps.tile([P, 1024], F32, tag="ps")
                    for (a, b) in _pieces(c0, 1024):
                        nc.tensor.matmul(
                            pscr[:, a:b],
                            kt_sb[poff:poff + 64, mcq, i * P:(i + 1) * P],
                            qt_sb[poff:poff + 64, mcq, hbase + a:hbase + b],
                            start=True, stop=True)
                    et = expp.tile([P, 1024], R32)
                    nc.scalar.activation(out=et[:, c0:1024],
                                         in_=pscr[:, c0:1024],
                                         func=mybir.ActivationFunctionType.Exp,
                                         scale=0.125)
                    if i * P >= hbase:  # diagonal block: zero sk > sq
                        nc.vector.tensor_mul(et[:, c0:c0 + P],
                                             et[:, c0:c0 + P], tri_sb)
                    for (a, b) in _pieces(c0, 1024):
                        nc.tensor.matmul(
                            pa[0:HD + 1, a:b],
                            v_sb[:, i, vlo:vlo + HD + 1],
                            et[:, a:b],
                            start=(i == 0), stop=(i == last_t[a // 512]))
                # normalize: row HD of pa is the softmax denominator
                drf = drp.tile([1, 2048], F32)
                dr = drp.tile([1, 1024], R32)
                nc.vector.tensor_copy(drf[:, 0:1024], pa[HD:HD + 1, :])
                nc.vector.reciprocal_approx_fast(out=drf[:, 1024:2048],
                                                 in_=drf[:, 0:1024])
                nc.vector.tensor_copy(dr[:, :], drf[:, 1024:2048])
                pb = ps.tile([64, 1024], F32, tag="ps")
                for (a, b) in _pieces(0, 1024):
                    nc.tensor.matmul(pb[:, a:b], ones_sb,
                                     dr[:, a:b],
                                     start=True, stop=True)
                asl = att_sb[poff:poff + 64, mcq, hbase:hbase + 1024]
                nc.vector.tensor_copy(asl, pa[0:64, :])
                nc.vector.tensor_mul(asl, asl, pb[:, :])

        # ---- output projection: out[s, :] = attT.T @ Wo_c ----
        for st in range(NST):
            po = ps.tile([P, 1024], F32, tag="ps")
            for mc2 in range(2):
                for (a, b) in _pieces(0, 1024):
                    nc.tensor.matmul(po[:, a:b],
                                     att_sb[:, mc2, st * P:(st + 1) * P],
                                     wo_sb[:, mc2, a:b],
                                     start=(mc2 == 0), stop=(mc2 == 1))
            ot = outp.tile([P, 1024], F32)
            nc.scalar.copy(ot[:, :], po[:, :])
            nc.sync.dma_start(out[st * P:(st + 1) * P, :], ot[:, :])

    nc.compile()
    return nc


def get_program():
    global _NC_CACHE
    if _NC_CACHE is None:
        _NC_CACHE = _build_program()
    return _NC_CACHE


def prepare_in_maps(inputs):
    x = np.asarray(inputs["x"], dtype=np.float32)
    Wq = np.asarray(inputs["Wq"], dtype=np.float32)
    Wk = np.asarray(inputs["Wk"], dtype=np.float32)
    Wv = np.asarray(inputs["Wv"], dtype=np.float32)
    Wo = np.asarray(inputs["Wo"], dtype=np.float32)
    xts = [np.ascontiguousarray(x[b].T) for b in range(B)]
    tri = np.triu(np.ones((P, P), dtype=np.float32))
    in_maps = []
    for c in range(NCORES):
        b = c // 4
        hg = c % 4
        cols = slice(hg * MC, (hg + 1) * MC)
        wv_c = np.zeros((D, VW), np.float32)
        for j in range(HC):
            wv_c[:, j * (HD + 1):j * (HD + 1) + HD] = \
                Wv[:, hg * MC + j * HD:hg * MC + (j + 1) * HD]
        in_maps.append({
            "xt": xts[b],
            "wq": np.ascontiguousarray(Wq[:, cols]),
            "wk": np.ascontiguousarray(Wk[:, cols]),
            "wv": wv_c,
            "wo": np.ascontiguousarray(Wo[cols, :]),
            "tri": tri,
        })
    return in_maps


def gather_output(results):
    outs = [np.asarray(results[c]["out"], dtype=np.float32)
            for c in range(NCORES)]
    return np.stack([outs[0] + outs[1] + outs[2] + outs[3],
                     outs[4] + outs[5] + outs[6] + outs[7]])


def kernel(**inputs) -> np.ndarray:
    nc = get_program()
    in_maps = prepare_in_maps(inputs)
    res = run_bass_kernel_spmd(nc, in_maps, list(range(NCORES)))
    return gather_output(res.results)


# revision 12
# speedup vs baseline: 1.0821x; 1.0221x over previous
"""Causal self-attention on 8 Trainium2 NeuronCores.

Sharding: batch (2) x head-groups (4 heads each) -> 8 cores. Each core
computes Q/K/V projections for its 4 heads, causal attention, and the
partial output projection for its head rows of Wo. The host sums the 4
partials per batch (the "all-reduce" of the row-sharded Wo done on host
during the gather step).

Device-side layout is fully transposed: QT/KT [m, s] come straight out of
W-stationary matmuls, scoresT [sk, sq] feed an augmented-V matmul whose
extra ones-column produces the softmax denominator for free, and the
normalized attendedT [m, s] is exactly the stationary operand the output
projection wants. The causal mask is applied as a multiplicative
upper-triangular 128x128 block on the diagonal score chunks; off-diagonal
masked chunks are never computed.

All matmul operands live in float32r (TF32-like, 1 PE cycle/row vs 4 for
fp32); PSUM accumulation stays fp32.
"""

from contextlib import ExitStack

import numpy as np

import concourse.bacc as bacc
import concourse.bass as bass  # noqa: F401  (AP helpers)
import concourse.mybir as mybir
import concourse.tile as tile
from concourse.bass_utils import run_bass_kernel_spmd

P = 128
B, S, D, H, HD = 2, 2048, 1024, 16, 64
NCORES = 8
HC = 4            # heads per core
MC = HC * HD      # 256 output columns (m) per core
VW = HC * (HD + 1)  # V'' width: 4 heads x (64 vals + 1 ones col)
NDC = D // P      # 8 contraction chunks
NST = S // P      # 16 sequence tiles
F32 = mybir.dt.float32
R32 = mybir.dt.float32r

_NC_CACHE = None


def _pieces(c0, c1, step=512):
    """Split [c0, c1) at `step`-aligned boundaries (PSUM-bank-safe matmuls)."""
    out = []
    c = c0
    while c < c1:
        n = min(c1, (c // step + 1) * step)
        out.append((c, n))
        c = n
    return out


def _build_program():
    nc = bacc.Bacc("TRN2", target_bir_lowering=False, debug=False)
    xt = nc.dram_tensor("xt", [D, S], R32, kind="ExternalInput").ap()
    wq = nc.dram_tensor("wq", [D, MC], R32, kind="ExternalInput").ap()
    wk = nc.dram_tensor("wk", [D, MC], R32, kind="ExternalInput").ap()
    wv = nc.dram_tensor("wv", [D, VW], R32, kind="ExternalInput").ap()
    wo = nc.dram_tensor("wo", [MC, D], R32, kind="ExternalInput").ap()
    tri = nc.dram_tensor("tri", [P, P], R32, kind="ExternalInput").ap()
    out = nc.dram_tensor("out", [S, D], F32, kind="ExternalOutput").ap()

    with tile.TileContext(nc) as tc, ExitStack() as ctx, \
            nc.allow_low_precision(reason="float32r matmul pipeline"):
        constp = ctx.enter_context(tc.tile_pool(name="constp", bufs=1))
        xtp = ctx.enter_context(tc.tile_pool(name="xtp", bufs=1))
        wp = ctx.enter_context(tc.tile_pool(name="wp", bufs=1))
        qkp = ctx.enter_context(tc.tile_pool(name="qkp", bufs=1))
        vp = ctx.enter_context(tc.tile_pool(name="vp", bufs=1))
        attp = ctx.enter_context(tc.tile_pool(name="attp", bufs=1))
        expp = ctx.enter_context(tc.tile_pool(name="expp", bufs=3))
        outp = ctx.enter_context(tc.tile_pool(name="outp", bufs=2))
        drp = ctx.enter_context(tc.tile_pool(name="drp", bufs=1))
        ps = ctx.enter_context(tc.tile_pool(name="ps", bufs=4, space="PSUM"))

        # constants: causal-keep mask (tri[r,c] = r<=c) + a ones row for the
        # denominator broadcast matmul, packed into one tile
        trio = constp.tile([P, P + 64], R32)
        nc.sync.dma_start(trio[:, 0:P], tri)
        # memset can't target f32r; write the 1.0f bit pattern as uint32
        ONE_BITS = 0x3F800000
        nc.vector.memset(trio[0:1, P:P + 64].bitcast(mybir.dt.uint32), ONE_BITS)
        tri_sb = trio[:, 0:P]
        ones_sb = trio[0:1, P:P + 64]

        # weights first (small), then x chunks — lets the first QKV matmuls
        # start as soon as xt chunk 0 lands instead of after the full 10.9MB
        wq_sb = wp.tile([P, NDC, MC], R32)
        wk_sb = wp.tile([P, NDC, MC], R32)
        wv_sb = wp.tile([P, NDC, VW], R32)
        wo_sb = wp.tile([P, 2, D], R32)
        for dc in range(NDC):
            nc.sync.dma_start(wq_sb[:, dc, :], wq[dc * P:(dc + 1) * P, :])
            nc.sync.dma_start(wk_sb[:, dc, :], wk[dc * P:(dc + 1) * P, :])
            nc.sync.dma_start(wv_sb[:, dc, :], wv[dc * P:(dc + 1) * P, :])
        xt_sb = xtp.tile([P, NDC, S], R32)
        for dc in range(NDC):
            nc.sync.dma_start(xt_sb[:, dc, :], xt[dc * P:(dc + 1) * P, :])
        for mc2 in range(2):
            nc.sync.dma_start(wo_sb[:, mc2, :], wo[mc2 * P:(mc2 + 1) * P, :])

        # ---- projections: QT/KT [m, s] (W stationary), V natural [s, m'] ----
        qt_sb = qkp.tile([P, 2, S], R32)
        kt_sb = qkp.tile([P, 2, S], R32)
        v_sb = vp.tile([P, NST, VW], R32)
        for mc2 in range(2):
            for slab in range(4):
                s0 = slab * 512
                pq = ps.tile([P, 512], F32, tag="ps")
                pk = ps.tile([P, 512], F32, tag="ps")
                for dc in range(NDC):
                    nc.tensor.matmul(pq[:, :],
                                     wq_sb[:, dc, mc2 * P:(mc2 + 1) * P],
                                     xt_sb[:, dc, s0:s0 + 512],
                                     start=(dc == 0), stop=(dc == NDC - 1))
                for dc in range(NDC):
                    nc.tensor.matmul(pk[:, :],
                                     wk_sb[:, dc, mc2 * P:(mc2 + 1) * P],
                                     xt_sb[:, dc, s0:s0 + 512],
                                     start=(dc == 0), stop=(dc == NDC - 1))
                nc.vector.tensor_copy(qt_sb[:, mc2, s0:s0 + 512], pq[:, :])
                nc.vector.tensor_copy(kt_sb[:, mc2, s0:s0 + 512], pk[:, :])
        for st in range(NST):
            pv = ps.tile([P, VW], F32, tag="ps")
            for dc in range(NDC):
                nc.tensor.matmul(pv[:, :],
                                 xt_sb[:, dc, st * P:(st + 1) * P],
                                 wv_sb[:, dc, :],
                                 start=(dc == 0), stop=(dc == NDC - 1))
            nc.vector.tensor_copy(v_sb[:, st, :], pv[:, :])
        for j in range(HC):
            nc.vector.memset(
                v_sb[:, :, j * (HD + 1) + HD].bitcast(mybir.dt.uint32), ONE_BITS)

        # ---- attention: two heads interleaved to keep the PE dense ----
        # (single-head chains stall the PE on the exp round-trip; the HAM
        # clock gate then never re-warms and the whole phase runs at 1.2GHz)
        att_sb = attp.tile([P, 2, S], R32)
        for mcq in range(2):
            for half in range(2):
                hbase = half * 1024
                nch = (half + 1) * 8  # causal: sk chunks 0 .. sq_max/128
                # last chunk touching each 512-col psum bank (for stop flags)
                last_t = {0: max(i for i in range(nch)
                                 if max(0, i * P - hbase) < 512),
                          1: nch - 1}
                pas = [ps.tile([P, 1024], F32, tag="ps", name=f"pa{s_}")
                       for s_ in range(2)]
                for i in range(nch):
                    c0 = max(0, i * P - hbase)  # first valid sq col (local)
                    for sub in range(2):
                        hh = 2 * mcq + sub
                        poff = sub * 64
                        vlo = hh * (HD + 1)
                        pa = pas[sub]
                        pscr = ps.tile([P, 1024], F32, tag="ps")
                        for (a, b) in _pieces(c0, 1024):
                            nc.tensor.matmul(
                                pscr[:, a:b],
                                kt_sb[poff:poff + 64, mcq, i * P:(i + 1) * P],
                                qt_sb[poff:poff + 64, mcq,
                                      hbase + a:hbase + b],
                                start=True, stop=True)
                        et = expp.tile([P, 1024], R32)
                        nc.scalar.activation(
                            out=et[:, c0:1024], in_=pscr[:, c0:1024],
                            func=mybir.ActivationFunctionType.Exp, scale=0.125)
                        if i * P >= hbase:  # diagonal block: zero sk > sq
                            nc.vector.tensor_mul(et[:, c0:c0 + P],
                                                 et[:, c0:c0 + P], tri_sb)
                        for (a, b) in _pieces(c0, 1024):
                            nc.tensor.matmul(
                                pa[0:HD + 1, a:b],
                                v_sb[:, i, vlo:vlo + HD + 1],
                                et[:, a:b],
                                start=(i == 0), stop=(i == last_t[a // 512]))
                # normalize: row HD of pa is the softmax denominator
                for sub in range(2):
                    poff = sub * 64
                    pa = pas[sub]
                    drf = drp.tile([1, 2048], F32)
                    dr = drp.tile([1, 1024], R32)
                    nc.vector.tensor_copy(drf[:, 0:1024], pa[HD:HD + 1, :])
                    nc.vector.reciprocal_approx_fast(out=drf[:, 1024:2048],
                                                     in_=drf[:, 0:1024])
                    nc.vector.tensor_copy(dr[:, :], drf[:, 1024:2048])
                    pb = ps.tile([64, 1024], F32, tag="ps")
                    for (a, b) in _pieces(0, 1024):
                        nc.tensor.matmul(pb[:, a:b], ones_sb, dr[:, a:b],
                                         start=True, stop=True)
                    asl = att_sb[poff:poff + 64, mcq, hbase:hbase + 1024]
                    nc.vector.tensor_copy(asl, pa[0:64, :])
                    nc.vector.tensor_mul(asl, asl, pb[:, :])

        # ---- output projection: out[s, :] = attT.T @ Wo_c ----
        for st in range(NST):
            po = ps.tile([P, 1024], F32, tag="ps")
            for mc2 in range(2):
                for (a, b) in _pieces(0, 1024):
                    nc.tensor.matmul(po[:, a:b],
                                     att_sb[:, mc2, st * P:(st + 1) * P],
                                     wo_sb[:, mc2, a:b],
                                     start=(mc2 == 0), stop=(mc2 == 1))
            ot = outp.tile([P, 1024], F32)
            nc.scalar.copy(ot[:, :], po[:, :])
            nc.sync.dma_start(out[st * P:(st + 1) * P, :], ot[:, :])

    nc.compile()
    return nc


def get_program():
    global _NC_CACHE
    if _NC_CACHE is None:
        _NC_CACHE = _build_program()
    return _NC_CACHE


def prepare_in_maps(inputs):
    x = np.asarray(inputs["x"], dtype=np.float32)
    Wq = np.asarray(inputs["Wq"], dtype=np.float32)
    Wk = np.asarray(inputs["Wk"], dtype=np.float32)
    Wv = np.asarray(inputs["Wv"], dtype=np.float32)
    Wo = np.asarray(inputs["Wo"], dtype=np.float32)
    xts = [np.ascontiguousarray(x[b].T) for b in range(B)]
    tri = np.triu(np.ones((P, P), dtype=np.float32))
    in_maps = []
    for c in range(NCORES):
        b = c // 4
        hg = c % 4
        cols = slice(hg * MC, (hg + 1) * MC)
        wv_c = np.zeros((D, VW), np.float32)
        for j in range(HC):
            wv_c[:, j * (HD + 1):j * (HD + 1) + HD] = \
                Wv[:, hg * MC + j * HD:hg * MC + (j + 1) * HD]
        in_maps.append({
            "xt": xts[b],
            "wq": np.ascontiguousarray(Wq[:, cols]),
            "wk": np.ascontiguousarray(Wk[:, cols]),
            "wv": wv_c,
            "wo": np.ascontiguousarray(Wo[cols, :]),
            "tri": tri,
        })
    return in_maps


def gather_output(results):
    outs = [np.asarray(results[c]["out"], dtype=np.float32)
            for c in range(NCORES)]
    return np.stack([outs[0] + outs[1] + outs[2] + outs[3],
                     outs[4] + outs[5] + outs[6] + outs[7]])


def kernel(**inputs) -> np.ndarray:
    nc = get_program()
    in_maps = prepare_in_maps(inputs)
    res = run_bass_kernel_spmd(nc, in_maps, list(range(NCORES)))
    return gather_output(res.results)
